# revision 1
# baseline (speedup 1.0000x reference)
"""BCJR decoder (rate-1/2 conv code, 64 states) on 8 Trainium2 cores.

Strategy
--------
Data-parallel over batch: 32 codewords per core. Within a core, each
codeword's T=2048 trellis steps are split into C=16 chunks of 128 steps,
decoded in parallel with L=32 warm-up steps on each side (windowed BCJR).
The time axis is padded with llr_a=+16 "pilot" steps which deterministically
collapse the state to 0, making chunk 0 / chunk 15 boundary conditions exact.

Layout: 128 SBUF partitions = 32 codewords x 4 chunk-groups; 4 more chunks
("f groups") along the free dimension. 192 sequential steps per pass.

Per step: PE matmul (sign-table [3,128] x llr triple) builds branch-metric
exponents E in PSUM; ScalarE exp(0.5 E) -> G; VectorE does the alpha/beta
recursions (gather-mul + pairwise-add via strided access patterns).
LLR output uses the identity  joint_b(t) = sum_{j in b-half} alpha'_{t+1}(j)
beta_{t+1}(j)  where alpha' is the pre-normalization forward pairsum.
"""

import os
from contextlib import ExitStack

import numpy as np

import concourse.bass as bass
import concourse.mybir as mybir
from concourse import tile as tile_mod
from concourse.tile_rust import add_dep_helper
from concourse.bass_utils import run_bass_kernel_spmd

# ---------------- problem constants (hardcoded) ----------------
B_FULL, N_FULL = 256, 4096
T = N_FULL // 2            # 2048 trellis steps
N_CORES = 8
B_CORE = B_FULL // N_CORES  # 32 codewords per core
C = 16                     # time chunks per codeword
S = T // C                 # 128 steps per chunk
L = 32                     # warmup steps each side
TL = S + 2 * L             # 192 local steps
CF = 4                     # chunks in free dim (C = 4 partition-groups * CF)
PAD_A = 16.0               # llr_a pad value (forces state collapse)
NORM_EVERY = 8

F32 = mybir.dt.float32
BF16 = mybir.dt.bfloat16


def _sign_table():
    """[3, 128] rows (la, l0, l1) x cols (b, s): E[s,b] = sum_c sign[c,(b,s)] * llr_c."""
    gen = ("1111001", "1011011")
    mu = 6
    g = np.array([[int(c) for c in p] for p in gen])
    opf = np.zeros((64, 2), np.int32)
    for s in range(64):
        rbits = [(s >> (mu - 1 - j)) & 1 for j in range(mu)]
        for b in range(2):
            w = np.array([b] + rbits)
            obits = (g @ w) % 2
            opf[s, b] = obits[0] * 2 + obits[1]
    ops = (1.0 - 2.0 * np.array([[(o >> (1 - j)) & 1 for j in range(2)]
                                 for o in range(4)])).astype(np.float32)
    sa = np.concatenate([np.ones(64), -np.ones(64)])
    s0 = np.concatenate([ops[opf[:, 0], 0], ops[opf[:, 1], 0]])
    s1 = np.concatenate([ops[opf[:, 0], 1], ops[opf[:, 1], 1]])
    return np.stack([sa, s0, s1]).astype(np.float32)  # [3, 128]


SIGN_NP = _sign_table()
# block-diag [12, 512]: rows (f*3+c), cols (f', (b,s))
SIGN_BD = np.zeros((12, 512), np.float32)
for _f in range(4):
    SIGN_BD[_f * 3:_f * 3 + 3, _f * 128:(_f + 1) * 128] = SIGN_NP

# ---------------- bass program ----------------
_NC_CACHE = {}


def _ap(a, offset_extra, dims):
    """Custom AP over the same tensor as `a` (partition dim kept)."""
    return bass.AP(tensor=a.tensor, offset=a.offset + offset_extra,
                   ap=[list(a.ap[0])] + [list(d) for d in dims])


def build_nc():
    nc = bass.Bass()
    llr_t_d = nc.declare_dram_parameter("llr_t", [12, TL * 128], F32, isOutput=False)
    sign_d = nc.declare_dram_parameter("sign", [12, 512], F32, isOutput=False)
    out_d = nc.declare_dram_parameter("llr_out", [B_CORE, T], F32, isOutput=True)
    dbg = os.environ.get("KDBG", "0") == "1"
    if dbg:
        dbg_jsum = nc.declare_dram_parameter("dbg_jsum", [128, S * 8], F32, isOutput=True)
        dbg_ahist = nc.declare_dram_parameter("dbg_ahist", [128, 2 * 256], F32, isOutput=True)
        dbg_g = nc.declare_dram_parameter("dbg_g", [128, 512], F32, isOutput=True)

    W = 24                 # llr_t streaming window (steps)
    NW = TL // W           # 6 windows
    mult = mybir.AluOpType.mult
    add = mybir.AluOpType.add

    with tile_mod.TileContext(nc) as tc, ExitStack() as ctx:
        # static ring buffers (pool alloc/release deps would exceed the
        # 1-sync-wait-per-instruction hardware limit)
        def ring(nm, n, shape, dt=F32):
            return [ctx.enter_context(nc.sbuf_tensor(f"{nm}{i}", shape, dt))
                    for i in range(n)]

        e_ps_bufs = [ctx.enter_context(nc.psum_tensor(f"eps{_i}", [128, 512], F32))
                     for _i in range(4)]
        g_bufs = ring("gbuf", 8, [128, 512], BF16)
        ag_bufs = ring("agbuf", 2, [128, 512], BF16)
        aw_bufs = ring("awbuf", 4, [128, 256], BF16)
        nrm_bufs = ring("nrmbuf", 2, [128, 2 * CF])
        jm_bufs = ring("jmbuf", 2, [128, 256], BF16)
        lt_bufs = ring("ltbuf", 2, [12, 24 * 128])

        dve_scr = ctx.enter_context(nc.sbuf_tensor("dvescr", [1, 8], F32))
        act_scr = ctx.enter_context(nc.sbuf_tensor("actscr", [1, 8], F32))
        act_scr2 = ctx.enter_context(nc.sbuf_tensor("actscr2", [1, 8], F32))
        sign_t = ctx.enter_context(nc.sbuf_tensor("sign_sb", [12, 512], F32))
        sign_sb = sign_t[:]
        nc.sync.dma_start(out=sign_sb, in_=sign_d[:])

        state = {"prev_g": None, "dve_stable": None}
        ahist_t = ctx.enter_context(nc.sbuf_tensor("ahist", [128, S * 256], BF16))
        ahist = ahist_t[:]
        jsum_t = ctx.enter_context(nc.sbuf_tensor("jsum", [128, S * 8], F32))
        jsum = jsum_t[:]
        _counters = {"g": 0, "ag": 0, "aw": 0, "nrm": 0, "jm": 0, "lt": 0}

        def nxt(nm, bufs):
            i = _counters[nm]
            _counters[nm] = i + 1
            return bufs[i % len(bufs)]

        def make_G(tau, lt_sb, fresh_dma, deint=False):
            """PE: E[row,(b,s)] per f into PSUM; ACT: G = exp(0.5 E).

            PE Matmult (LW struct) supports only ONE sync wait, so 1-element
            dummy matmuls absorb the PSUM-WAR and window-DMA waits first.
            """
            e_ps = e_ps_bufs[tau % 4]
            col = (tau % W) * 128
            nc.tensor.matmul(out=e_ps[0:1, 0:1], lhsT=sign_t[0:1, 0:1],
                             rhs=sign_t[0:1, 0:1], start=True, stop=True)
            if fresh_dma:
                nc.tensor.matmul(out=e_ps[0:1, 0:1],
                                 lhsT=lt_sb[0:1, col:col + 1],
                                 rhs=sign_t[0:1, 0:1], start=True, stop=True)
            nc.tensor.matmul(
                out=e_ps[:], lhsT=lt_sb[:, col:col + 128],
                rhs=sign_sb, start=True, stop=True)
            g_sb = nxt("g", g_bufs)[:]
            q = tau % 8
            # ACT absorber chain (HW: one sync wait per instruction):
            #   anchor: self-wait on previous exp (keeps wm fresh)
            #   c2a:    DVE wait covering the g-slot WAR (reads a stable
            #           DVE output newer than the old g reader)
            #   exp:    carries only the PE wait
            prev_g = state.get("prev_g")
            anchor_src = prev_g if prev_g is not None else sign_sb
            i_anchor = nc.scalar.copy(out=act_scr[0:1, q:q + 1],
                                      in_=anchor_src[0:1, 0:1])
            dsrc = state.get("dve_stable")
            c2a_src = dsrc if dsrc is not None else sign_sb
            i_c2a = nc.scalar.copy(out=act_scr2[0:1, q:q + 1],
                                   in_=c2a_src[0:1, 0:1])
            if deint:
                g_out = _ap(g_sb, 0, [[128, 4], [64, 2], [1, 32], [32, 2]])
            else:
                g_out = g_sb.rearrange("p (f b s) -> p f b s", f=CF, b=2)
            i_exp = nc.scalar.activation(
                out=g_out,
                in_=e_ps[:].rearrange("p (f b s) -> p f b s", f=CF, b=2),
                func=mybir.ActivationFunctionType.Exp, scale=0.5)
            add_dep_helper(i_c2a.ins, i_anchor.ins, False, "act-order")
            add_dep_helper(i_exp.ins, i_c2a.ins, False, "act-order")
            state["prev_g"] = g_sb
            return g_sb

        def load_window(w):
            lt_sb = nxt("lt", lt_bufs)
            nc.sync.dma_start(out=lt_sb[:],
                              in_=llr_t_d[:, w * W * 128:(w + 1) * W * 128])
            return lt_sb

        def normalize(cur):
            nb = nxt("nrm", nrm_bufs)
            asum = nb[:, 0:CF]
            nc.vector.tensor_reduce(
                out=asum, in_=cur.rearrange("p (f s) -> p f s", f=CF),
                axis=mybir.AxisListType.X, op=add)
            rz = nb[:, CF:2 * CF]
            nc.vector.reciprocal(out=rz, in_=asum)
            anorm = nxt("aw", aw_bufs)[:]
            rz_b = _ap(rz, 0, [[1, CF], [0, 64]])
            nc.vector.tensor_tensor(
                out=anorm.rearrange("p (f s) -> p f s", f=CF),
                in0=cur.rearrange("p (f s) -> p f s", f=CF),
                in1=rz_b, op=mult)
            return anorm

        # ---------------- forward ----------------
        alpha = nxt("aw", aw_bufs)[:]
        nc.vector.memset(alpha, 1.0 / 64)
        lt_sb = None
        for tau in range(TL):
            fresh = tau % W == 0
            if fresh:
                lt_sb = load_window(tau // W)
            state["dve_stable"] = (
                ahist[:, (tau - 8 - L) * 256:(tau - 8 - L) * 256 + 1]
                if L + 8 <= tau < L + S + 8 else alpha)
            g_sb = make_G(tau, lt_sb, fresh)
            ag = nxt("ag", ag_bufs)[:]
            # DVE absorber: self-wait on prev pairsum, so ag carries only ACT
            i_d1 = nc.vector.tensor_copy(
                out=dve_scr[0:1, tau % 8:tau % 8 + 1], in_=alpha[0:1, 0:1])
            a_b = _ap(alpha, 0, [[64, CF], [0, 2], [1, 64]])
            i_ag = nc.vector.tensor_tensor(
                out=ag.rearrange("p (f b s) -> p f b s", f=CF, b=2),
                in0=g_sb.rearrange("p (f b s) -> p f b s", f=CF, b=2),
                in1=a_b, op=mult)
            add_dep_helper(i_ag.ins, i_d1.ins, False, "dve-order")
            # pairsum -> alpha' (prenorm); store to ahist when in output range
            if L <= tau < L + S:
                dst = ahist[:, (tau - L) * 256:(tau - L + 1) * 256]
            else:
                dst = nxt("aw", aw_bufs)[:]
            ev = _ap(ag, 0, [[128, CF], [64, 2], [2, 32]])
            od = _ap(ag, 1, [[128, CF], [64, 2], [2, 32]])
            nc.vector.tensor_tensor(
                out=dst.rearrange("p (f b k) -> p f b k", f=CF, b=2),
                in0=ev, in1=od, op=add)
            alpha = dst
            if tau % NORM_EVERY == NORM_EVERY - 1:
                alpha = normalize(alpha)

        # ---------------- backward ----------------
        beta = nxt("aw", aw_bufs)[:]
        nc.vector.memset(beta, 1.0 / 64)
        for tau in range(TL - 1, -1, -1):
            fresh = tau % W == W - 1
            if fresh:
                lt_sb = load_window(tau // W)
            if L <= tau < L + S:
                k = tau - L
                jm = nxt("jm", jm_bufs)[:]
                nc.vector.tensor_tensor(
                    out=jm, in0=ahist[:, k * 256:(k + 1) * 256],
                    in1=beta, op=mult)
                nc.vector.tensor_reduce(
                    out=jsum[:, k * 8:(k + 1) * 8].rearrange(
                        "p (f b) -> p f b", b=2),
                    in_=jm.rearrange("p (f b k) -> p f b k", f=CF, b=2),
                    axis=mybir.AxisListType.X, op=add)
            state["dve_stable"] = beta
            g_sb = make_G(tau, lt_sb, fresh, deint=True)
            bg = nxt("ag", ag_bufs)[:]
            i_d1 = nc.vector.tensor_copy(
                out=dve_scr[0:1, tau % 8:tau % 8 + 1], in_=beta[0:1, 0:1])
            # layout (f, b, m, k): all operands unit-stride innermost (2x mode)
            b_g = _ap(beta, 0, [[64, CF], [32, 2], [0, 2], [1, 32]])
            g_in = _ap(g_sb, 0, [[128, CF], [64, 2], [32, 2], [1, 32]])
            bg_out = _ap(bg, 0, [[128, CF], [64, 2], [32, 2], [1, 32]])
            i_bg = nc.vector.tensor_tensor(out=bg_out, in0=g_in, in1=b_g,
                                           op=mult)
            add_dep_helper(i_bg.ins, i_d1.ins, False, "dve-order")
            dst = nxt("aw", aw_bufs)[:]
            # iterate (f, m, k): out idx 64f + 2k + m ; bg idx 128f + b64 + 32m + k
            lo = _ap(bg, 0, [[128, CF], [32, 2], [1, 32]])
            hi = _ap(bg, 64, [[128, CF], [32, 2], [1, 32]])
            nc.vector.tensor_tensor(
                out=_ap(dst, 0, [[64, CF], [1, 2], [2, 32]]),
                in0=lo, in1=hi, op=add)
            beta = dst
            if tau % NORM_EVERY == 0:
                beta = normalize(beta)

        if dbg:
            nc.sync.dma_start(out=dbg_jsum[:], in_=jsum)
            nc.sync.dma_start(out=dbg_ahist[:, 0:256], in_=ahist[:, 0:256])
            nc.sync.dma_start(out=dbg_ahist[:, 256:512], in_=ahist[:, 127 * 256:128 * 256])
            nc.sync.dma_start(out=dbg_g[:], in_=g_bufs[0][:])
        # ---------------- epilogue: llr = ln(j0) - ln(j1) ----------------
        lg_t = ctx.enter_context(nc.sbuf_tensor("lgbuf", [128, S * 8], F32))
        lg = lg_t[:]
        nc.scalar.activation(out=lg, in_=jsum,
                             func=mybir.ActivationFunctionType.Ln)
        llr_t2 = ctx.enter_context(nc.sbuf_tensor("llrsb", [128, 512], F32))
        llr_sb = llr_t2
        in0 = _ap(lg, 0, [[2, CF], [8, S]])
        in1 = _ap(lg, 1, [[2, CF], [8, S]])
        nc.vector.tensor_tensor(
            out=llr_sb[:].rearrange("p (f k) -> p f k", f=CF),
            in0=in0, in1=in1, op=mybir.AluOpType.subtract)
        # out DMA: partition p = cw*4+g holds (f,k); chunk = g*4+f
        # single fused DMA: src iterates (p=(cw,g), f, k); dst (cw, g, f, k)
        src_ap = llr_sb[:].rearrange("p (f k) -> p f k", f=4)
        dst_ap = bass.AP(tensor=out_d[:].tensor, offset=0,
                         ap=[[2048, 32], [512, 4], [128, 4], [1, 128]])
        nc.sync.dma_start(out=dst_ap, in_=src_ap)
    return nc


_ENG_SELF = {"PE": "PE_", "DVE": "DVE_", "Activation": "Activation_",
             "Pool": "Pool_", "SP": "SP_"}


def _prune_waits(nc):
    """Drop sem waits already implied, so each instruction carries <=1.

    HW structs accept one sync wait per instruction. Tile emits waits that
    are provably satisfied at issue. Vector-clock rules:
      - cross-engine sems: knowledge from transitive joins of kept waits
      - self sems (same engine): only monotone vs explicitly-waited values
        (ACT/DVE completion is not implied by issue order); PE and DMA
        queues complete in order, so own-increment knowledge counts there.
    """
    know = {}        # proc -> {sem_id: known completed value}
    waited_max = {}  # proc -> {sem_id: max explicitly waited}
    sem_total = {}   # sem_id -> running total
    hist = {}        # sem_id -> [(total_after, snapshot)]
    out_dma_sems = set()
    bad = []
    for b in nc.m.functions[0].blocks:
        for i in b.instructions:
            si = i.sync_info
            op = str(getattr(i, "opcode", type(i).__name__))
            if si is None:
                continue
            upds = [u for u in (si.on_update or [])
                    if u.sync_type == "semaphore"
                    and u.update_mode in ("sem-inc", "sem-add-imm")]
            if "DMACopy" in op and upds:
                proc = str(upds[0].ant_name)
                outs = getattr(i, "outs", None) or []
                if outs and "llr_out" in str(getattr(outs[0], "memref", "")):
                    out_dma_sems.add(upds[0].id)
            else:
                proc = getattr(i.engine, "value", str(i.engine))
            k = know.setdefault(proc, {})
            wm = waited_max.setdefault(proc, {})
            in_order = proc == "PE" or proc.startswith("DMAHW")
            if "Drain" in op and si.on_wait and len(si.on_wait) > 1:
                si.on_wait = [w for w in si.on_wait if w.id in out_dma_sems]
                continue
            skip = ("Drain" in op) or ("EventSem" in op)
            ow = list(si.on_wait or [])
            if ow and not skip:
                keep = []
                for w in ow:
                    if (w.sync_type != "semaphore"
                            or w.wait_mode != "sem-ge-imm"
                            or w.wait_value is None
                            or str(w.ant_name).startswith("barrier")):
                        keep.append(w)
                        continue
                    v = w.wait_value
                    nm = str(w.ant_name)
                    is_self = nm == proc or nm.startswith(proc + "_")
                    if is_self:
                        implied = (wm.get(w.id, -1) >= v
                                   or (in_order and k.get(w.id, 0) >= v))
                    else:
                        implied = (k.get(w.id, 0) >= v
                                   or wm.get(w.id, -1) >= v)
                    if implied:
                        continue
                    keep.append(w)
                    wm[w.id] = max(wm.get(w.id, -1), v)
                    for tot, snap in hist.get(w.id, ()):
                        if tot >= v:
                            for s2, v2 in snap.items():
                                if k.get(s2, 0) < v2:
                                    k[s2] = v2
                            break
                    if k.get(w.id, 0) < v:
                        k[w.id] = v
                if len(keep) != len(ow):
                    si.on_wait = keep
                    ow = keep
                if len(ow) > 1:
                    bad.append((i.name, op,
                                [(x.ant_name, x.wait_value) for x in ow]))
            for u in upds:
                tot = sem_total.get(u.id, 0) + (u.update_value or 0)
                sem_total[u.id] = tot
                k[u.id] = tot
                hist.setdefault(u.id, []).append((tot, dict(k)))
    if bad:
        raise RuntimeError(f"{len(bad)} insts still multi-wait: {bad[:8]}")
    return nc


def _get_nc():
    if "nc" not in _NC_CACHE:
        _NC_CACHE["nc"] = _prune_waits(build_nc())
    return _NC_CACHE["nc"]


# ---------------- host-side layout ----------------
def _prep_core(llr_ch_c, llr_a_c):
    """llr_ch_c [32, 4096], llr_a_c [32, 2048] -> llr_t [12, TL*128] f32."""
    lc = np.zeros((B_CORE, T + 2 * L, 2), np.float32)
    lc[:, L:L + T] = llr_ch_c.reshape(B_CORE, T, 2)
    la = np.full((B_CORE, T + 2 * L), PAD_A, np.float32)
    la[:, L:L + T] = llr_a_c
    # windows [B, C, TL, comp]
    idx = (np.arange(C)[:, None] * S + np.arange(TL)[None, :])  # [C, TL]
    w = np.stack([la[:, idx], lc[:, idx, 0], lc[:, idx, 1]], -1)  # [B, C, TL, 3]
    # chunk c = g*4+f ; row = cw*4+g ; llr_t[f*3+comp, tau*128+row]
    w = w.reshape(B_CORE, 4, 4, TL, 3)            # [cw, g, f, tau, comp]
    w = w.transpose(2, 4, 3, 0, 1)                # [f, comp, tau, cw, g]
    return np.ascontiguousarray(w.reshape(12, TL * 128))


def _run(llr_ch, llr_a, trace=False):
    nc = _get_nc()
    in_maps = []
    for core in range(N_CORES):
        sl = slice(core * B_CORE, (core + 1) * B_CORE)
        in_maps.append({
            "llr_t": _prep_core(np.asarray(llr_ch[sl], np.float32),
                                np.asarray(llr_a[sl], np.float32)),
            "sign": SIGN_BD,
        })
    res = run_bass_kernel_spmd(nc, in_maps, core_ids=list(range(N_CORES)),
                               trace=trace)
    out = np.concatenate([r["llr_out"] for r in res.results], 0)
    return out.astype(np.float32), res


def kernel(llr_ch, llr_a):
    out, _ = _run(llr_ch, llr_a, trace=False)
    return out

